# revision 33
# baseline (speedup 1.0000x reference)
"""Trainium2 Bass/Tile kernel: symmetric contrastive loss (CLIP-style).

Distribution: data-parallel over B across 8 NeuronCores.  Each core MLPs +
l2-normalizes its 2048-row shard of both branches, AllGathers the normalized
num-projections (bf16, 512KB/rank), computes its row-block of the 16384^2
logit matrix tile-by-tile (never materialized), and reduces rows (i2n) and
columns (n2i partials, AllReduce-add at the end).

Local rows are processed in a p-major permutation (partition p holds DRAM
rows p*16..p*16+15) so every input DMA is 16-32KB-contiguous per partition
(descriptor-light).  The same permutation applies to both branches, so
diagonal pairs still line up, and the loss is a mean over rows/cols so the
permutation is otherwise invisible.

The exp of 33.5M logits/core is the wall, so row-chunks are split between:
  * A rows (11/16): ACT Exp with fused accum_out -> row sums; PE ones-matmul
    accumulates column sums in PSUM.
  * D rows (5/16): Schraudolph exp on the Vector engine -- tensor_scalar
    computes x*128/ln2 + magic into int16 whose bit pattern IS bf16 exp(x)
    (max 4.2% elementwise noise, mean-calibrated under 0.2%; harmless after
    16K-element sums), then one fused custom-DVE op adds e into a column
    accumulator and folds a row-sum (telescoped) in the same pass.

Logits are bounded (|cos|/temp <= 10) so no max shift is needed.  The l2
normalization is exp(-0.5*ln(|z|^2) - 0.5*log_temp) with Identity/Square
work on DVE so ACT needs only Relu/Ln/Exp (~3 activation-table loads); the
num branch normalizes half its rows at a time so each AllGather half
launches as early as possible.
"""

import numpy as np

N_CORES = 8
B = 16384
D_IMG = 2048
D_NUM = 256
P = 128

# Schraudolph constants (bf16-target): int16 bits = x*128/ln2 + SCH_B.
# SCH_B sits midway between the truncate (16249.15) and round-to-nearest
# (16248.65) calibrations so either convert behavior keeps the mean
# multiplicative bias of exp under 0.2%.
SCH_A = 128.0 / float(np.log(2.0))
SCH_B = 16248.90

_NC_CACHE = {}
_DVE_OPS = {}


def _register_dve_ops():
    """Register the two fused DVE ops used by the D-path (runtime append to
    dve_ops.OPS; sha computed from lower() so the pin always matches)."""
    if _DVE_OPS:
        return _DVE_OPS
    from concourse.dve_ops import DveOp, OPS, CUSTOM_DVE_SPECS, _SUB_OPCODE_FOR_NAME
    from concourse.dve_spec import Spec, Src0, Src1, Zero, lower
    from concourse.dve_spec import _has_src1 as has_src1
    from concourse.dve_uop import DveOpSpec
    from operator import add

    def _ref_sum(body_fn):
        def _r(in0, in1, c0, c1, c2):
            b = body_fn(in0, in1, c0, c1, c2).astype(np.float32)
            return b, b.reshape(b.shape[0], -1).sum(axis=-1, keepdims=True)
        return _r

    defs = [
        ("EXPACC_SUM_ANT",
         Spec(body=Src0 + Src1, accum=add,
              reference=_ref_sum(lambda in0, in1, c0, c1, c2:
                                 in0.astype(np.float32) + in1))),
        ("EXPCPY_SUM_ANT",
         Spec(body=Src0 + Zero, accum=add,
              reference=_ref_sum(lambda in0, in1, c0, c1, c2:
                                 in0.astype(np.float32)))),
    ]
    for name, spec in defs:
        if name in _SUB_OPCODE_FOR_NAME:
            op = next(o for o in OPS if o.name == name)
            _DVE_OPS[name] = op
            continue
        row = max(_SUB_OPCODE_FOR_NAME.values()) + 1
        assert row < 0x20, "custom-DVE row field overflow"
        _SUB_OPCODE_FOR_NAME[name] = row
        shas = {}
        for ver in ("v3", "v4"):
            s = DveOpSpec(name=name, opcode=row, uops=lower(spec, ver=ver),
                          rd1_en=has_src1(spec))
            shas[ver] = s.sha(ver)
        op = DveOp(name, spec, subdim=False, uops_sha=shas)
        OPS.append(op)
        CUSTOM_DVE_SPECS[name] = spec
        _DVE_OPS[name] = op
    return _DVE_OPS


def build(b_total=B, d_img=D_IMG, d_num=D_NUM, n_cores=N_CORES):
    """Build + compile the Bass module. Returns the compiled Bacc object."""
    key = (b_total, d_img, d_num, n_cores)
    if key in _NC_CACHE:
        return _NC_CACHE[key]

    import concourse.bacc as bacc
    import concourse.mybir as mybir
    import concourse.tile as tile

    ops = _register_dve_ops()
    EXPACC = ops["EXPACC_SUM_ANT"]
    EXPCPY = ops["EXPCPY_SUM_ANT"]

    dt = mybir.dt
    AF = mybir.ActivationFunctionType
    Alu = mybir.AluOpType
    AX = mybir.AxisListType
    f32 = dt.float32
    bf16 = dt.bfloat16
    i16 = dt.int16

    BL = b_total // n_cores          # local rows per core
    assert BL % 512 == 0 and b_total % 1024 == 0
    NRT = BL // 512                  # 512-wide row tiles (MLP / transpose)
    NRC = BL // 128                  # 128-row chunks (main pass)
    GPP = BL // P                    # rows per partition in p-major layout
    KI = d_img // 128                # contraction tiles, img MLP1
    KN = d_num // 128
    CW = 1024                        # main-pass column supertile width
    NCT = b_total // CW
    NH = CW // 512
    ARW = b_total + 64               # AllReduce payload width

    # D-path (Vector-engine exp) row chunks, interleaved among ACT chunks.
    D_RC = [rc for rc in (2, 5, 8, 11, 14) if rc < NRC]
    if not D_RC:
        D_RC = [NRC - 1]
    D_SET = set(D_RC)
    D_IDX = {rc: i for i, rc in enumerate(D_RC)}
    A_RC = [rc for rc in range(NRC) if rc not in D_SET]
    A_IDX = {rc: i for i, rc in enumerate(A_RC)}
    ND, NA = len(D_RC), len(A_RC)

    nc = bacc.Bacc("TRN2", target_bir_lowering=False, debug=False,
                   num_devices=n_cores)

    img = nc.dram_tensor("img_feat", [BL, d_img], f32, kind="ExternalInput").ap()
    num = nc.dram_tensor("num_feat", [BL, d_num], f32, kind="ExternalInput").ap()
    Wi1 = nc.dram_tensor("Wi1", [d_img, P], f32, kind="ExternalInput").ap()
    bi1 = nc.dram_tensor("bi1", [P, 1], f32, kind="ExternalInput").ap()
    Wi2 = nc.dram_tensor("Wi2", [P, P], f32, kind="ExternalInput").ap()
    bi2 = nc.dram_tensor("bi2", [P, 1], f32, kind="ExternalInput").ap()
    Wn1 = nc.dram_tensor("Wn1", [d_num, P], f32, kind="ExternalInput").ap()
    bn1 = nc.dram_tensor("bn1", [P, 1], f32, kind="ExternalInput").ap()
    Wn2 = nc.dram_tensor("Wn2", [P, P], f32, kind="ExternalInput").ap()
    bn2 = nc.dram_tensor("bn2", [P, 1], f32, kind="ExternalInput").ap()
    ltm = nc.dram_tensor("log_temp", [1, 1], f32, kind="ExternalInput").ap()
    loss = nc.dram_tensor("loss", [1, 1], f32, kind="ExternalOutput").ap()

    rg = [list(range(n_cores))]

    # p-major row views: partition p holds DRAM rows p*GPP .. p*GPP+GPP-1,
    # so each partition's slice is one contiguous DRAM chunk.
    img_pm = img.rearrange("(p g) e -> p g e", p=P)
    num_pm = num.rearrange("(p g) e -> p g e", p=P)

    with tile.TileContext(nc) as tc:
        with (
            tc.tile_pool(name="sb", bufs=1) as sb,
            tc.tile_pool(name="stream", bufs=3) as st,
            tc.tile_pool(name="vstage", bufs=2) as vs,
            tc.tile_pool(name="dram", bufs=1, space="DRAM") as dram,
        ):
            xsp_pool = tc.tile_pool(name="xsp", bufs=2)
            xsp = xsp_pool.__enter__()
            xtp_pool = tc.tile_pool(name="xtp", bufs=2)
            xtp = xtp_pool.__enter__()
            nin_pool = tc.tile_pool(name="nin", bufs=1)
            nin = nin_pool.__enter__()

            # num input first on the sync queue -- it gates the AllGather
            # chain; p-major makes it 128 x 16KB descriptors (~7us).
            xs_n = nin.tile([P, NRC // 4 * 4, d_num], f32)
            nc.sync.dma_start(xs_n[:], num_pm)

            # weights + biases follow on the sync queue.  All of these are
            # small; the big img stream is gated until they have landed,
            # because its 32KB packets starve any concurrent small DMA.
            wn1_f = sb.tile([P, KN * P], f32)
            nc.sync.dma_start(wn1_f.rearrange("p (k m) -> p k m", k=KN),
                              Wn1.rearrange("(k p) m -> p k m", p=P))
            wn2_f = sb.tile([P, P], f32)
            nc.sync.dma_start(wn2_f[:], Wn2)
            bn1_sb = sb.tile([P, 1], f32)
            nc.sync.dma_start(bn1_sb[:], bn1)
            bn2_sb = sb.tile([P, 1], f32)
            nc.sync.dma_start(bn2_sb[:], bn2)
            bi1_sb = sb.tile([P, 1], f32)
            nc.sync.dma_start(bi1_sb[:], bi1)
            bi2_sb = sb.tile([P, 1], f32)
            nc.sync.dma_start(bi2_sb[:], bi2)
            lt_sb = sb.tile([1, 1], f32)
            nc.sync.dma_start(lt_sb[:], ltm)

            # ---------------- constants ----------------
            ones_kb = sb.tile([P, 1], bf16)
            nc.vector.memset(ones_kb[:], 1.0)
            ones_kf = sb.tile([P, 1], f32)
            nc.vector.memset(ones_kf[:], 1.0)
            ones_1f = sb.tile([1, P], f32)
            nc.vector.memset(ones_1f[:], 1.0)
            zpad = sb.tile([1, 64], f32)
            nc.vector.memset(zpad[:], 0.0)
            idn_i = sb.tile([P, P], dt.int32)
            nc.gpsimd.iota(idn_i[:], pattern=[[1, P]], base=0,
                           channel_multiplier=-1)
            idn = sb.tile([P, P], bf16)
            nc.vector.tensor_scalar(idn[:], idn_i[:], 0, None,
                                    op0=Alu.is_equal)
            idn_f = sb.tile([P, P], f32)
            nc.vector.tensor_scalar(idn_f[:], idn_i[:], 0, None,
                                    op0=Alu.is_equal)
            wn1_sb = sb.tile([P, KN * P], bf16)
            nc.vector.tensor_copy(wn1_sb[:], wn1_f[:])
            wn2_sb = sb.tile([P, P], bf16)
            nc.vector.tensor_copy(wn2_sb[:], wn2_f[:])
            nhlt = sb.tile([1, 1], f32)        # -0.5 * log_temp
            nc.vector.tensor_scalar_mul(nhlt[:], lt_sb[:], -0.5)

            # img weights load together with the other small DMAs (they are
            # descriptor-heavy and would starve under the img stream).
            wi1_sb = sb.tile([P, KI * P], bf16)
            nc.gpsimd.dma_start(wi1_sb.rearrange("p (k m) -> p k m", k=KI),
                                Wi1.rearrange("(k p) m -> p k m", p=P))
            wi2_sb = sb.tile([P, P], bf16)
            nc.gpsimd.dma_start(wi2_sb[:], Wi2)

            # The 16MB img stream is held until every small load has landed:
            # its 32KB packets starve any concurrent small DMA.  The
            # scheduler orders only by data dependencies, so each xs tile
            # gets a dummy write of the gate value before its DMA (WAW
            # ordering) and the gate reads a slice of every small load.
            gate = sb.tile([1, 4], f32)
            nc.gpsimd.tensor_copy(gate[:], xs_n[0:1, 0, 0:4])
            nc.gpsimd.tensor_copy(gate[:1, 0:1], wn1_f[0:1, 0:1])
            nc.gpsimd.tensor_copy(gate[:1, 1:2], bi2_sb[0:1, :])
            nc.gpsimd.tensor_copy(gate[:1, 2:3], lt_sb[:1, :])
            nc.gpsimd.tensor_copy(gate[:1, 3:4], wi1_sb[0:1, 0:1])

            def gated_img_dma(rb):
                xs = xsp.tile([P, 4, d_img], bf16, tag="xsi", name="xsi")
                nc.gpsimd.tensor_copy(xs[0:1, 0, 0:4], gate[:1, :])
                nc.gpsimd.dma_start(xs[:], img_pm[:, rb * 4:(rb + 1) * 4, :])
                return xs

            xs_list = []
            for rb in range(min(2, NRT)):
                xs_list.append(gated_img_dma(rb))

            # ---------------- DRAM scratch ----------------
            BH = BL // 2
            ag_in_a = dram.tile([P, BH], bf16)
            ag_in_b = dram.tile([P, BH], bf16)
            ag_out_a = dram.tile([n_cores * P, BH], bf16, addr_space="Shared")
            ag_out_b = dram.tile([n_cores * P, BH], bf16, addr_space="Shared")
            ARH = (NCT - 2) * CW     # tail AllReduce carries 2 supertiles
            ar_in = dram.tile([1, ARW], f32)
            wu_in = dram.tile([1, 16], f32)
            wu_out = dram.tile([8, 16], f32, addr_space="Shared")
            ar_out_a = dram.tile([1, ARH], f32, addr_space="Shared")
            ar_out_b = dram.tile([1, ARW - ARH], f32, addr_space="Shared")

            # ---------------- persistent SBUF ----------------
            xnT = sb.tile([P, KN * BL], bf16)   # num input, transposed
            h1n = sb.tile([P, BL], bf16)
            h1i = sb.tile([P, BL], bf16)
            zn = sb.tile([P, BL], bf16)
            zi = sb.tile([P, BL], bf16)
            ntl = sb.tile([P, BL], bf16)        # normalized num proj (local)
            itl = sb.tile([P, BL], bf16)        # normalized img proj (local)
            npf = sb.tile([P, b_total], bf16)   # gathered num proj (all cores)
            rowacc = sb.tile([P, NA * NCT], f32)
            sacc = sb.tile([P, ND * NCT], f32)
            dsum = sb.tile([1, 1], f32)         # running sum of diag
            nc.vector.memset(dsum[:], 0.0)
            vrow = sb.tile([1, BL], f32)

            # warm-up collective: absorbs the ~20-40us first-collective
            # setup cost of the CC stream while the prologue DMAs run.
            nc.sync.dma_start(wu_in[:], zpad[:1, 0:16])
            nc.gpsimd.collective_compute(
                "AllGather", Alu.bypass, replica_groups=rg,
                ins=[wu_in.opt()], outs=[wu_out.opt()])

            def mlp2_norm(pp, h1, w2, b2, z, outp, rts, after=None):
                """z = w2.T@h1 + b2; outp = z * inv with
                inv = exp(-0.5*ln(|z|^2) - 0.5*log_temp), for row tiles in
                `rts`.  Bias-add and squaring run on DVE; only Ln/Exp
                (batched over `rts`) touch ACT."""
                lo, hi = rts[0] * 512, (rts[-1] + 1) * 512
                for rt in rts:
                    sl = slice(rt * 512, (rt + 1) * 512)
                    pz = pp.tile([P, 512], f32, tag="zb", name="pz")
                    nc.tensor.matmul(pz[:], w2[:], h1[:, sl])
                    nc.vector.tensor_scalar(z[:, sl], pz[:], b2[:], None,
                                            op0=Alu.add)
                    sq = st.tile([P, 512], bf16, tag="sq", name="sq")
                    nc.vector.tensor_mul(sq[:], z[:, sl], z[:, sl])
                    pv = pp.tile([P, 512], f32, tag="v", name="pv")
                    nc.tensor.matmul(pv[:1, :], ones_kb[:], sq[:])
                    nc.vector.tensor_copy(vrow[:1, sl], pv[:1, :])
                lnv = vs.tile([1, BL], f32, tag="lnv", name="lnv", bufs=1)
                nc.scalar.activation(lnv[:1, lo:hi], vrow[:1, lo:hi], AF.Ln)
                inv = vs.tile([1, BL], f32, tag="inv", name="inv", bufs=1)
                nc.scalar.activation(inv[:1, lo:hi], lnv[:1, lo:hi], AF.Exp,
                                     bias=nhlt[:], scale=-0.5)
                for rt in rts:
                    sl = slice(rt * 512, (rt + 1) * 512)
                    pb = pp.tile([P, 512], f32, tag="zb", name="pb")
                    nc.tensor.matmul(pb[:], ones_1f[:], inv[:1, sl])
                    nc.vector.tensor_mul(outp[:, sl], z[:, sl], pb[:])
                if after is not None:
                    after()

            # ---------------- num branch + AllGather ----------------
            npf_v = npf.rearrange("p (r c) -> p r c", c=BL)

            def trigger_ag_a():
                nc.sync.dma_start(ag_in_a[:], ntl[:, 0:BH])
                nc.gpsimd.collective_compute(
                    "AllGather", Alu.bypass, replica_groups=rg,
                    ins=[ag_in_a.opt()], outs=[ag_out_a.opt()])

            def trigger_ag_b():
                nc.sync.dma_start(ag_in_b[:], ntl[:, BH:BL])
                nc.gpsimd.collective_compute(
                    "AllGather", Alu.bypass, replica_groups=rg,
                    ins=[ag_in_b.opt()], outs=[ag_out_b.opt()])
                # gathered halves land in SBUF as they complete; these waits
                # sit at the tail of the sync queue so they don't stall the
                # second AllGather's input copy.
                nc.sync.dma_start(npf_v[:, :, 0:BH],
                                  ag_out_a.rearrange("(r p) n -> p r n", p=P))
                nc.sync.dma_start(npf_v[:, :, BH:BL],
                                  ag_out_b.rearrange("(r p) n -> p r n", p=P))

            with tc.tile_pool(name="pp1", bufs=2, space="PSUM") as pp:
                for dk in range(KN):
                    for gb in range(NRC // 4):
                        pt = pp.tile([P, 512], f32, tag="pt", name="ptn")
                        for q in range(4):
                            nc.tensor.transpose(
                                pt[:, q * P:(q + 1) * P],
                                xs_n[:, gb * 4 + q, dk * P:(dk + 1) * P],
                                idn_f[:])
                        nc.vector.tensor_copy(
                            xnT[:, dk * BL + gb * 512: dk * BL + gb * 512 + 512],
                            pt[:])
                for rt in range(NRT):
                    sl = slice(rt * 512, (rt + 1) * 512)
                    ph = pp.tile([P, 512], f32, tag="h", name="ph")
                    for k in range(KN):
                        nc.tensor.matmul(
                            ph[:], wn1_sb[:, k * P:(k + 1) * P],
                            xnT[:, k * BL + rt * 512: k * BL + rt * 512 + 512],
                            start=(k == 0), stop=(k == KN - 1))
                    nc.scalar.activation(h1n[:, sl], ph[:], AF.Relu, bias=bn1_sb[:])
                half = max(1, NRT // 2)
                mlp2_norm(pp, h1n, wn2_sb, bn2_sb, zn, ntl,
                          list(range(half)), after=trigger_ag_a)
                if half < NRT:
                    mlp2_norm(pp, h1n, wn2_sb, bn2_sb, zn, ntl,
                              list(range(half, NRT)), after=trigger_ag_b)
                else:
                    trigger_ag_b()
            nin_pool.__exit__(None, None, None)

            # remaining img tiles (same gating)
            for rb in range(min(2, NRT), NRT):
                xs_list.append(gated_img_dma(rb))

            # ---------------- img branch ----------------
            with tc.tile_pool(name="pp2", bufs=2, space="PSUM") as pp:
                for rb in range(NRT):
                    rsl = slice(rb * 512, (rb + 1) * 512)
                    xs = xs_list[rb]
                    xtb = xtp.tile([P, KI * 512], bf16, tag="xt", name="xtb")
                    for dk in range(KI):
                        pt = pp.tile([P, 512], bf16, tag="pt", name="pt")
                        for q in range(4):
                            nc.tensor.transpose(
                                pt[:, q * P:(q + 1) * P],
                                xs[:, q, dk * P:(dk + 1) * P], idn[:])
                        nc.vector.tensor_copy(
                            xtb[:, dk * 512:(dk + 1) * 512], pt[:])
                    ph = pp.tile([P, 512], f32, tag="h", name="phi")
                    for k in range(KI):
                        nc.tensor.matmul(
                            ph[:], wi1_sb[:, k * P:(k + 1) * P],
                            xtb[:, k * 512:(k + 1) * 512],
                            start=(k == 0), stop=(k == KI - 1))
                    nc.scalar.activation(h1i[:, rsl], ph[:], AF.Relu,
                                         bias=bi1_sb[:])
                mlp2_norm(pp, h1i, wi2_sb, bi2_sb, zi, itl, list(range(NRT)))
                # diagonal: l_ii = sum_p itl[p,i] * ntl[p,i]; accumulate sum
                for rt in range(NRT):
                    sl = slice(rt * 512, (rt + 1) * 512)
                    prod = st.tile([P, 512], bf16, tag="sq", name="prod")
                    nc.vector.tensor_mul(prod[:], itl[:, sl], ntl[:, sl])
                    pd = pp.tile([P, 512], f32, tag="h", name="pd")
                    nc.tensor.matmul(pd[:1, :], ones_kb[:], prod[:])
                    dred = vs.tile([1, 1], f32, tag="dred", name="dred")
                    nc.vector.reduce_sum(dred[:], pd[:1, :], axis=AX.X)
                    nc.vector.tensor_add(dsum[:], dsum[:], dred[:])
            xtp_pool.__exit__(None, None, None)
            xsp_pool.__exit__(None, None, None)

            # ---------------- main pass ----------------
            ct_a = [ct for ct in range(NCT)
                    if (ct * CW) % BL + CW <= BH]
            ct_order = ct_a + [ct for ct in range(NCT) if ct not in ct_a]
            NHALF = NCT // 2
            with (
                tc.tile_pool(name="pl", bufs=3, space="PSUM") as plp,
                tc.tile_pool(name="pc", bufs=1, space="PSUM") as pcp,
                tc.tile_pool(name="epool", bufs=4) as ep,
                tc.tile_pool(name="accp", bufs=2) as ap_,
                tc.tile_pool(name="eip", bufs=2) as eip,
            ):
                for pos, ct in enumerate(ct_order):
                    pcol = pcp.tile([P, CW], f32, tag="pc", name="pcol")
                    acc_prev = None
                    first_col = True
                    for rc in range(NRC):
                        plog = plp.tile([P, CW], f32, tag="pl", name="plog")
                        for h in range(NH):
                            nc.tensor.matmul(
                                plog[:, h * 512:(h + 1) * 512],
                                itl[:, rc * P:(rc + 1) * P],
                                npf[:, ct * CW + h * 512: ct * CW + (h + 1) * 512])
                        if rc in D_SET:
                            # DVE: Schraudolph exp (int16 bits = bf16 e)
                            ei = eip.tile([P, CW], i16, tag="ei", name="ei")
                            nc.vector.tensor_scalar(
                                ei[:], plog[:], SCH_A, SCH_B,
                                op0=Alu.mult, op1=Alu.add)
                            ev = ei[:].bitcast(bf16)
                            slot = D_IDX[rc] * NCT + ct
                            acc = ap_.tile([P, CW], bf16, tag="acc", name="acc")
                            if acc_prev is None:
                                nc.vector._custom_dve(
                                    EXPCPY, out=acc[:], in0=ev,
                                    accum_out=sacc[:, slot:slot + 1])
                            else:
                                nc.vector._custom_dve(
                                    EXPACC, out=acc[:], in0=ev, in1=acc_prev[:],
                                    accum_out=sacc[:, slot:slot + 1])
                            acc_prev = acc
                        else:
                            e = ep.tile([P, CW], bf16, tag="e", name="e")
                            slot = A_IDX[rc] * NCT + ct
                            nc.scalar.activation(
                                e[:], plog[:], AF.Exp,
                                accum_out=rowacc[:, slot:slot + 1])
                            for h in range(NH):
                                nc.tensor.matmul(
                                    pcol[:1, h * 512:(h + 1) * 512],
                                    ones_kb[:], e[:, h * 512:(h + 1) * 512],
                                    start=first_col, stop=False)
                            first_col = False
                    for h in range(NH):
                        nc.tensor.matmul(
                            pcol[:1, h * 512:(h + 1) * 512],
                            ones_kb[:], acc_prev[:, h * 512:(h + 1) * 512],
                            start=False, stop=True)
                    cst = vs.tile([1, CW], f32, tag="cst", name="cst")
                    nc.vector.tensor_copy(cst[:], pcol[:1, :])
                    nc.sync.dma_start(ar_in[:1, pos * CW:(pos + 1) * CW], cst[:])
                    if pos == NCT - 3:
                        nc.gpsimd.collective_compute(
                            "AllReduce", Alu.add, replica_groups=rg,
                            ins=[ar_in[:1, 0:ARH].opt()], outs=[ar_out_a.opt()])

                # ---- row direction partials ----
                rs_all = sb.tile([P, NRC], f32)
                nc.vector.reduce_sum(
                    rs_all[:, 0:NA],
                    rowacc.rearrange("p (a ct) -> p a ct", ct=NCT), axis=AX.X)
                dsd = sb.tile([P, ND * NCT], f32)
                nc.vector.tensor_copy(dsd[:, 0:NCT], sacc[:, 0:NCT])
                if ND > 1:
                    nc.vector.tensor_sub(dsd[:, NCT:], sacc[:, NCT:],
                                         sacc[:, 0:(ND - 1) * NCT])
                nc.vector.reduce_sum(
                    rs_all[:, NA:NRC],
                    dsd.rearrange("p (d ct) -> p d ct", ct=NCT), axis=AX.X)
                lse_r = sb.tile([P, NRC], f32)
                lsum = sb.tile([P, 1], f32)
                nc.scalar.activation(lse_r[:], rs_all[:], AF.Ln,
                                     accum_out=lsum[:])
                pR = pcp.tile([P, CW], f32, tag="pc", name="pR")
                nc.tensor.matmul(pR[:1, :1], ones_kf[:], lsum[:])
                rpart = sb.tile([1, 1], f32)
                nc.vector.tensor_sub(rpart[:], pR[:1, :1], dsum[:])
                nc.sync.dma_start(ar_in[:1, b_total:b_total + 1], rpart[:])
                nc.sync.dma_start(ar_in[:1, b_total + 1:b_total + 2], dsum[:])
                nc.sync.dma_start(ar_in[:1, b_total + 2:ARW], zpad[:1, :62])

                # ---- AllReduce (second-half columns + scalars) ----
                nc.gpsimd.collective_compute(
                    "AllReduce", Alu.add, replica_groups=rg,
                    ins=[ar_in[:1, ARH:ARW].opt()], outs=[ar_out_b.opt()])

                # first-half column lse (ar_out_a landed long ago)
                csb_a = sb.tile([P, ARH // P], f32)
                nc.sync.dma_start(
                    csb_a[:],
                    ar_out_a.rearrange("o (a b) -> (o a) b", a=P))
                lse_ca = sb.tile([P, ARH // P], f32)
                csum_a = sb.tile([P, 1], f32)
                nc.scalar.activation(lse_ca[:], csb_a[:], AF.Ln,
                                     accum_out=csum_a[:])

                # ---- final ----
                HB2 = b_total - ARH
                csb_b = sb.tile([P, HB2 // P], f32)
                nc.sync.dma_start(
                    csb_b[:],
                    ar_out_b[:1, :HB2].rearrange("o (a b) -> (o a) b", a=P))
                sc2 = sb.tile([1, 2], f32)
                nc.sync.dma_start(sc2[:], ar_out_b[:1, HB2:HB2 + 2])
                lse_cb = sb.tile([P, HB2 // P], f32)
                csum_p = sb.tile([P, 1], f32)
                nc.scalar.activation(lse_cb[:], csb_b[:], AF.Ln,
                                     accum_out=csum_p[:])
                nc.vector.tensor_add(csum_p[:], csum_p[:], csum_a[:])
                pC = pcp.tile([P, CW], f32, tag="pc", name="pC")
                nc.tensor.matmul(pC[:1, :1], ones_kf[:], csum_p[:])
                t1 = sb.tile([1, 1], f32)
                nc.vector.tensor_add(t1[:], pC[:1, :1], sc2[:1, 0:1])
                t2 = sb.tile([1, 1], f32)
                nc.vector.tensor_sub(t2[:], t1[:], sc2[:1, 1:2])
                lsb = sb.tile([1, 1], f32)
                nc.vector.tensor_scalar_mul(lsb[:], t2[:], 1.0 / (2.0 * b_total))
                nc.sync.dma_start(loss, lsb[:])

    nc.compile()
    _NC_CACHE[key] = nc
    return nc


def shard_inputs(inputs, b_total=B, n_cores=N_CORES):
    BL = b_total // n_cores
    img = np.ascontiguousarray(np.asarray(inputs["img_feat"], dtype=np.float32))
    num = np.ascontiguousarray(np.asarray(inputs["num_feat"], dtype=np.float32))

    def mat(name):
        return np.ascontiguousarray(np.asarray(inputs[name], dtype=np.float32))

    def col(name):
        return np.ascontiguousarray(
            np.asarray(inputs[name], dtype=np.float32).reshape(P, 1))

    lt = np.asarray(inputs["log_temp"], dtype=np.float32).reshape(1, 1)
    shared = {
        "Wi1": mat("Wi1"), "Wi2": mat("Wi2"),
        "Wn1": mat("Wn1"), "Wn2": mat("Wn2"),
        "bi1": col("bi1"), "bi2": col("bi2"),
        "bn1": col("bn1"), "bn2": col("bn2"),
        "log_temp": np.ascontiguousarray(lt),
    }
    maps = []
    for c in range(n_cores):
        m = dict(shared)
        m["img_feat"] = np.ascontiguousarray(img[c * BL:(c + 1) * BL])
        m["num_feat"] = np.ascontiguousarray(num[c * BL:(c + 1) * BL])
        maps.append(m)
    return maps


def run(inputs, trace=False, **kw):
    """Run on hardware; returns (loss_scalar, BassKernelResults)."""
    from concourse.bass_utils import run_bass_kernel_spmd
    nc = build()
    res = run_bass_kernel_spmd(nc, shard_inputs(inputs),
                               core_ids=list(range(N_CORES)), trace=trace, **kw)
    val = np.asarray(res.results[0]["loss"], dtype=np.float32).reshape(())
    return val, res


def kernel(**inputs):
    val, _ = run(inputs)
    return val
